# revision 1
# baseline (speedup 1.0000x reference)
"""Trainium2 Bass kernel for nn_Caps1D (capsule routing, 3 iterations).

Sharding: pure data-parallel over batch B=1024 across 8 cores (128/core).
W is replicated. Output [1024, 2] gathered from per-core [128, 2].

Math restructure (per core, per class k):
  u_ji[b,r,o] = sum_i u[b,r,i] W[k,r,i,o]           (never materialized)
  t=1: c uniform ->  s~1[b,o] = sum_j u[b,j] W2[j,o]      (PE, contraction j=(r,i))
  squash via gamma: v = gamma(n) * s,  n=|s|^2, gamma=sqrt(n)/(1+n)
  delta[b,r] = alpha[b] * sum_i u[b,(r,i)] * Ws[b,(r,i)],  Ws = outer(s_norm, WoT)
  L += alpha * delta_pre ; c~ = exp(L) (unnormalized, Z via ACT accum)
  x~ = u * c~ (broadcast over i) ; s~ = sum_j x~T[j,b] W2[j,o]  (PE)
  classes_k = n3/(1+n3); out = softmax_k(classes)

Layout: contraction index j uses "block-i-major" order within 128-chunks:
  j = 128*c + 32*i + rl  where r = 32*c + rl
so segment-reduce over i and the c~ broadcast are stride-1 on DVE while the
W DMA access pattern stays linear.

Engine balance: PE does all matmuls/transposes; ACT does PSUM evacuations +
exp; DVE does the elementwise muls/reduces (bf16, 2x mode). k=0's Ws is
consumed straight from PSUM by DVE while k=1's is evacuated by ACT, so the
two class pipelines use complementary engines and overlap.
"""

import numpy as np

import bass_rust
import concourse.bass as bass
import concourse.mybir as mybir
from concourse import tile
from concourse.bass_utils import run_bass_kernel_spmd

# problem dims (hardcoded per contest rules)
B, R, Cin, K, Cout = 1024, 2336, 4, 2, 16
NCORES = 8
BL = B // NCORES          # 128 batch rows per core
RI = R * Cin              # 9344
NCH = RI // 128           # 73 contraction chunks
RL = 32                   # routes per chunk
KO = K * Cout             # 32
CW = 64                   # padded chunk-col stride: col = CW*c + 32*k + o

F32 = mybir.dt.float32
BF16 = mybir.dt.bfloat16
AF = mybir.ActivationFunctionType
OP = mybir.AluOpType


def _split_ctrl_waits(nc, max_waits=1):
    """walrus here rejects >1 sync-wait per instruction; hoist extras onto
    single-wait NoOps inserted just before (same engine, program order)."""
    for fn in nc.m.functions:
        for bb in fn.blocks:
            out, changed = [], False
            for ins in bb.instructions:
                si = ins.sync_info
                if (
                    si is not None
                    and si.on_wait is not None
                    and len(si.on_wait) > max_waits
                ):
                    waits = list(si.on_wait)
                    for j, w in enumerate(waits[:-1]):
                        out.append(
                            mybir.InstNoOp(
                                name=f"{ins.name}-waitsplit-{j}",
                                engine=ins.engine,
                                ins=[],
                                outs=[],
                                sync_info=bass_rust.SyncInfo(on_wait=[w], on_update=[]),
                            )
                        )
                    ins.sync_info = bass_rust.SyncInfo(
                        on_wait=[waits[-1]], on_update=list(si.on_update or [])
                    )
                    changed = True
                out.append(ins)
            if changed:
                bb.instructions = out


def _psum_chunks():
    out, j = [], 0
    while j < RI:
        sz = min(512, RI - j)
        out.append((j, sz))
        j += sz
    return out


def build_nc(debug=(), nrep=1):
    nc = bass.Bass()
    u_d = nc.declare_dram_parameter("u", [BL, R, Cin], F32, isOutput=False)
    w_d = nc.declare_dram_parameter("W", [K, R, Cin, Cout], F32, isOutput=False)
    out_d = nc.declare_dram_parameter("out", [BL, K], F32, isOutput=True)
    dbg_d = {
        name: nc.declare_dram_parameter(name, shape, F32, isOutput=True)
        for name, shape in debug
    }

    with tile.TileContext(nc) as tc:
        with (
            tc.tile_pool(name="big", bufs=1) as big,
            tc.tile_pool(name="small", bufs=1) as small,
            tc.tile_pool(name="ps", bufs=2, space=bass.MemorySpace.PSUM) as ps,
            tc.tile_pool(name="pst", bufs=2, space=bass.MemorySpace.PSUM) as pst,
        ):
            # ---------- persistent SBUF tiles ----------
            # sh1 slot (37.4KB): u_f32 at startup, then per-iteration k=1
            # scratch (ws1 -> xmod1). sh0 slot (18.7KB): k=0 ws0 -> xmod0.

            u_im = big.tile([128, RI], BF16, tag="u_im")
            xT = big.tile([128, RI], BF16, tag="xT")
            w2p = big.tile([128, NCH * CW], BF16, tag="w2p")
            wot = big.tile([64, RI], BF16, tag="wot")
            Lk = [big.tile([128, R], F32, name=f"Lk{k}", tag=f"L{k}") for k in range(K)]
            ct = [big.tile([128, R], BF16, name=f"ct{k}", tag=f"ct{k}") for k in range(K)]
            dtmp0 = big.tile([128, R], BF16, tag="dtmp0")
            dpre = big.tile([128, R], BF16, tag="dpre")

            iota32 = small.tile([128, 128], mybir.dt.int32, tag="iota")
            id_bf = small.tile([128, 128], BF16, tag="id_bf")
            id_f32 = small.tile([128, 128], F32, tag="id_f32")
            stp = small.tile([64, 128], F32, tag="stp")
            s_nrm = small.tile([128, 64], BF16, tag="s_nrm")
            sTn = small.tile([64, 128], BF16, tag="sTn")
            Zk = [small.tile([128, 2], F32, name=f"Zk{k}", tag=f"Z{k}") for k in range(K)]
            Zs = [small.tile([128, 1], F32, name=f"Zs{k}", tag=f"Zs{k}") for k in range(K)]
            rZ = small.tile([128, K], F32, tag="rZ")
            sq = small.tile([128, 64], F32, tag="sq")
            nval = small.tile([128, 4], F32, tag="nval")
            lnn = small.tile([128, 4], F32, tag="lnn")
            tau = small.tile([128, 4], F32, tag="tau")
            onepn = small.tile([128, 4], F32, tag="onepn")
            ripn = small.tile([128, 4], F32, tag="ripn")
            alpha = small.tile([128, 4], F32, tag="alpha")
            cls = small.tile([128, K], F32, tag="cls")
            clse = small.tile([128, K], F32, tag="clse")
            clsum = small.tile([128, 1], F32, tag="clsum")
            rcs = small.tile([128, 1], F32, tag="rcs")
            outt = small.tile([128, K], F32, tag="outt")

            chunks = _psum_chunks()

            def emit_body(rep):
                u_f32 = big.tile([128, RI], F32, name=f"uf32_{rep}", tag="sh1")
                w2p_f32 = big.tile(
                    [128, NCH * CW], F32, name=f"w2pf_{rep}", tag="sh0"
                )
                # ---------- identities first (gate all PE transposes) ------
                nc.gpsimd.iota(
                    iota32[:], pattern=[[1, 128]], base=0, channel_multiplier=-1
                )
                nc.vector.tensor_scalar(id_bf[:], iota32[:], 0, None, op0=OP.is_equal)
                nc.vector.tensor_scalar(id_f32[:], iota32[:], 0, None, op0=OP.is_equal)
                for k in range(K):
                    nc.gpsimd.memset(Lk[k][:], 0.0)

                # ---------- W2p load (HWDGE ring 2, fp32) + cast ----------
                nc.gpsimd.memset(w2p_f32[:], 0.0)
                for i in range(Cin):
                    for k in range(K):
                        dst = w2p_f32[32 * i : 32 * (i + 1), :].rearrange(
                            "rl (c g o) -> g rl c o", c=NCH, g=2 * K, o=Cout
                        )[2 * k]
                        src = w_d[k, :, i, :].rearrange("(c rl) o -> rl c o", rl=RL)
                        nc.scalar.dma_start(out=dst, in_=src)
                nc.vector.tensor_copy(w2p[:], w2p_f32[:])

                # ---------- u load + u_im build, sliced for pipelining -----
                uflat = u_d[:].rearrange("b r i -> b (r i)")
                uiv = u_im[:].rearrange("b (c i rl) -> b c i rl", c=NCH, i=Cin, rl=RL)
                ufv = u_f32[:].rearrange("b (c rl i) -> b c i rl", c=NCH, rl=RL, i=Cin)
                for si, c0 in enumerate(range(0, NCH, 8)):
                    cn = min(8, NCH - c0)
                    eng = nc.sync if si % 2 == 0 else nc.scalar
                    eng.dma_start(
                        out=u_f32[:, 128 * c0 : 128 * (c0 + cn)],
                        in_=uflat[:, 128 * c0 : 128 * (c0 + cn)],
                    )
                    nc.scalar.copy(out=uiv[:, c0 : c0 + cn], in_=ufv[:, c0 : c0 + cn])

                # ---------- uT chunks + s~1T accumulation + WoT ------------
                s1ps = pst.tile([64, 128], F32, tag="spsacc")
                for c0 in range(0, NCH, 8):
                    gn = min(8, NCH - c0)
                    tp = ps.tile([128, 1024], BF16, tag="tp_xt")
                    for g in range(gn):
                        nc.tensor.transpose(
                            tp[:, 128 * g : 128 * (g + 1)],
                            u_im[:, 128 * (c0 + g) : 128 * (c0 + g + 1)],
                            id_bf[:],
                        )
                    if (c0 // 8) % 2 == 0:
                        nc.scalar.copy(
                            out=xT[:, 128 * c0 : 128 * (c0 + gn)], in_=tp[:, : 128 * gn]
                        )
                    else:
                        nc.vector.tensor_copy(
                            xT[:, 128 * c0 : 128 * (c0 + gn)], tp[:, : 128 * gn]
                        )
                # s1 matmuls after all transposes: they gate on the W load,
                # which lands later than u -- emitting them afterward keeps
                # the PE FIFO from head-of-line blocking on w2p
                for c in range(NCH):
                    nc.tensor.matmul(
                        s1ps[:],
                        w2p[:, CW * c : CW * (c + 1)],
                        xT[:, 128 * c : 128 * (c + 1)],
                        start=(c == 0),
                        stop=(c == NCH - 1),
                    )
                for c0 in range(0, NCH, 8):
                    gn = min(8, NCH - c0)
                    tpw = ps.tile([128, 1024], BF16, tag="tp_xt")
                    for g in range(gn):
                        nc.tensor.transpose(
                            tpw[:64, 128 * g : 128 * (g + 1)],
                            w2p[:, CW * (c0 + g) : CW * (c0 + g + 1)],
                            id_bf[:],
                        )
                    if (c0 // 8) % 2 == 0:
                        nc.scalar.copy(
                            out=wot[:, 128 * c0 : 128 * (c0 + gn)],
                            in_=tpw[:64, : 128 * gn],
                        )
                    else:
                        nc.vector.tensor_copy(
                            wot[:, 128 * c0 : 128 * (c0 + gn)], tpw[:64, : 128 * gn]
                        )

                def squash_k(t, k, sps):
                    """Per-class squash from s~T psum -> sTn_k slice (+alpha
                    or classes). sps rows [32k:32k+16] hold s~T for class k."""
                    nc.scalar.copy(
                        out=stp[32 * k : 32 * k + 16, :],
                        in_=sps[32 * k : 32 * k + 16, :],
                    )
                    tp = pst.tile([128, 128], F32, tag="tp_small", bufs=1)
                    nc.tensor.transpose(tp[:, :64], stp[:], id_f32[:64, :64])
                    kk = slice(2 * k, 2 * k + 1)
                    if t == 1:
                        zs = 1.0 / R
                    else:
                        nc.vector.tensor_add(
                            Zs[k][:], Zk[k][:, 0:1], Zk[k][:, 1:2]
                        )
                        nc.vector.reciprocal(rZ[:, k : k + 1], Zs[k][:])
                        zs = rZ[:, k : k + 1]
                    # normalized s (bf16, for sTn) + fused square/accum -> n
                    nc.vector.tensor_scalar_mul(
                        s_nrm[:, 32 * k : 32 * k + 16],
                        tp[:, 32 * k : 32 * k + 16],
                        zs,
                    )
                    nc.scalar.activation(
                        sq[:, 32 * k : 32 * k + 16],
                        tp[:, 32 * k : 32 * k + 16],
                        AF.Square,
                        scale=zs,
                        accum_out=nval[:, kk],
                    )
                    nc.scalar.activation(onepn[:, kk], nval[:, kk], AF.Identity, bias=1.0)
                    nc.vector.reciprocal(ripn[:, kk], onepn[:, kk])
                    if t < 3:
                        nc.scalar.activation(lnn[:, kk], nval[:, kk], AF.Ln)
                        nc.scalar.activation(tau[:, kk], lnn[:, kk], AF.Exp, scale=0.5)
                        nc.vector.tensor_mul(alpha[:, kk], tau[:, kk], ripn[:, kk])
                        # sTn rows [32k:32k+16] <- transpose of full s_nrm
                        tp2 = pst.tile([128, 128], BF16, tag="tp_small", bufs=1)
                        nc.tensor.transpose(tp2[:64, :128], s_nrm[:], id_bf[:])
                        nc.scalar.copy(
                            out=sTn[32 * k : 32 * k + 16, :],
                            in_=tp2[32 * k : 32 * k + 16, :128],
                        )
                    else:
                        nc.vector.tensor_mul(
                            cls[:, k : k + 1], nval[:, kk], ripn[:, kk]
                        )


                for k in range(K):
                    squash_k(1, k, s1ps)

                ws = {}
                xmod = {}

                def stA(t, ks):
                    """Ws outer-product chunks; k=0 consumed by DVE straight
                    from PSUM (-> m0), k=1 evacuated by ACT. When both
                    classes requested, alternate per chunk so DVE and ACT
                    consumers run concurrently."""
                    for k in ks:
                        tag = "sh0" if k == 0 else "sh1"
                        ws[(t, k)] = big.tile(
                            [128, RI], BF16, name=f"ws{k}_{rep}_{t}", tag=tag
                        )
                    for (j0, sz) in chunks:
                        for k in ks:
                            wps = ps.tile([128, 512], F32, tag="wps", bufs=3)
                            nc.tensor.matmul(
                                wps[:, :sz],
                                sTn[32 * k : 32 * k + 16, :],
                                wot[32 * k : 32 * k + 16, j0 : j0 + sz],
                                start=True,
                                stop=True,
                            )
                            if k == 0:
                                nc.vector.tensor_mul(
                                    ws[(t, 0)][:, j0 : j0 + sz],
                                    u_im[:, j0 : j0 + sz],
                                    wps[:, :sz],
                                )
                            else:
                                nc.scalar.copy(
                                    out=ws[(t, 1)][:, j0 : j0 + sz], in_=wps[:, :sz]
                                )

                CH = 32  # half boundary in chunks (group-aligned)

                def stB(t, k):
                    """delta segreduce + L update + exp + x~ modulation, in
                    two j-halves so downstream transposes start earlier."""
                    tag = "sh0" if k == 0 else "sh1"
                    xmod[(t, k)] = big.tile(
                        [128, RI], BF16, name=f"xm{k}_{rep}_{t}", tag=tag
                    )
                    mv = ws[(t, k)][:].rearrange(
                        "b (c i rl) -> b i c rl", c=NCH, i=Cin
                    )
                    d0v = dtmp0[:].rearrange("b (c rl) -> b c rl", c=NCH)
                    dpv = dpre[:].rearrange("b (c rl) -> b c rl", c=NCH)
                    uiv2 = u_im[:].rearrange("b (c i rl) -> b c i rl", c=NCH, i=Cin)
                    cbv = (
                        ct[k][:]
                        .rearrange("b (c rl) -> b c rl", c=NCH)
                        .unsqueeze(2)
                        .broadcast_to([128, NCH, Cin, RL])
                    )
                    for h, (ca, cb_) in enumerate(((0, CH), (CH, NCH))):
                        ja, jb = 128 * ca, 128 * cb_
                        ra, rb = RL * ca, RL * cb_
                        if k == 1:
                            nc.vector.tensor_mul(
                                ws[(t, 1)][:, ja:jb],
                                u_im[:, ja:jb],
                                ws[(t, 1)][:, ja:jb],
                            )
                        nc.vector.tensor_add(
                            d0v[:, ca:cb_], mv[:, 0, ca:cb_], mv[:, 1, ca:cb_]
                        )
                        nc.vector.tensor_add(
                            dpv[:, ca:cb_], mv[:, 2, ca:cb_], mv[:, 3, ca:cb_]
                        )
                        nc.vector.tensor_add(
                            dpre[:, ra:rb], dtmp0[:, ra:rb], dpre[:, ra:rb]
                        )
                        nc.vector.scalar_tensor_tensor(
                            out=Lk[k][:, ra:rb],
                            in0=dpre[:, ra:rb],
                            scalar=alpha[:, 2 * k : 2 * k + 1],
                            in1=Lk[k][:, ra:rb],
                            op0=OP.mult,
                            op1=OP.add,
                        )
                        nc.scalar.activation(
                            ct[k][:, ra:rb],
                            Lk[k][:, ra:rb],
                            AF.Exp,
                            accum_out=Zk[k][:, h : h + 1],
                        )
                        nc.vector.tensor_mul(
                            xmod[(t, k)][:, ja:jb].rearrange(
                                "b (c i rl) -> b c i rl", c=cb_ - ca, i=Cin
                            ),
                            uiv2[:, ca:cb_],
                            cbv[:, ca:cb_],
                        )

                def stC(t, k, split_evac=False):
                    """x~T transposes + s~T accumulation; returns psum acc."""
                    sps = pst.tile([64, 128], F32, name=f"sps{t}{k}", tag="spsacc")
                    for c0 in range(0, NCH, 8):
                        gn = min(8, NCH - c0)
                        tp = ps.tile([128, 1024], BF16, tag="tp_xt")
                        for g in range(gn):
                            nc.tensor.transpose(
                                tp[:, 128 * g : 128 * (g + 1)],
                                xmod[(t, k)][:, 128 * (c0 + g) : 128 * (c0 + g + 1)],
                                id_bf[:],
                            )
                        if split_evac and (c0 // 8) % 2 == 1:
                            nc.vector.tensor_copy(
                                xT[:, 128 * c0 : 128 * (c0 + gn)].bitcast(
                                    mybir.dt.int32
                                ),
                                tp[:, : 128 * gn].bitcast(mybir.dt.int32),
                            )
                        else:
                            nc.scalar.copy(
                                out=xT[:, 128 * c0 : 128 * (c0 + gn)],
                                in_=tp[:, : 128 * gn],
                            )
                        for g in range(gn):
                            c = c0 + g
                            nc.tensor.matmul(
                                sps[:],
                                w2p[:, CW * c : CW * (c + 1)],
                                xT[:, 128 * c : 128 * (c + 1)],
                                start=(c == 0),
                                stop=(c == NCH - 1),
                            )
                    return sps

                # software-pipelined emission: t+1 stage A of class k starts
                # right after class k's squash, overlapping the other class's
                # stage C on complementary engines.
                stA(2, (0, 1))
                stB(2, 0); stB(2, 1)
                sps20 = stC(2, 0); squash_k(2, 0, sps20)
                stA(3, (0,))
                sps21 = stC(2, 1); squash_k(2, 1, sps21)
                stA(3, (1,))
                stB(3, 0); stB(3, 1)
                sps30 = stC(3, 0); squash_k(3, 0, sps30)
                sps31 = stC(3, 1, split_evac=True); squash_k(3, 1, sps31)

                # out = softmax over k of classes
                nc.scalar.activation(clse[:], cls[:], AF.Exp)
                nc.vector.tensor_add(clsum[:], clse[:, 0:1], clse[:, 1:2])
                nc.vector.reciprocal(rcs[:], clsum[:])
                nc.vector.tensor_scalar_mul(outt[:], clse[:], rcs[:])
                nc.sync.dma_start(out=out_d[:], in_=outt[:])

                for name, _ in debug:
                    srcs = {"dbg_L0": Lk[0], "dbg_L1": Lk[1], "dbg_cls": cls,
                            "dbg_alpha": alpha}[name]
                    nc.sync.dma_start(out=dbg_d[name][:], in_=srcs[:])

            for _rep in range(nrep):
                emit_body(_rep)

    _split_ctrl_waits(nc)
    return nc


_CACHED = {}


def _get_nc(debug=(), nrep=1):
    key = (tuple(debug), nrep)
    if key not in _CACHED:
        _CACHED[key] = build_nc(debug, nrep=nrep)
    return _CACHED[key]


def kernel(u: np.ndarray, W: np.ndarray, debug=(), trace=False):
    u = np.ascontiguousarray(u, dtype=np.float32)
    W = np.ascontiguousarray(W, dtype=np.float32)
    assert u.shape == (B, R, Cin) and W.shape == (K, R, Cin, Cout)
    nc = _get_nc(debug)
    in_maps = [
        {"u": u[i * BL : (i + 1) * BL], "W": W} for i in range(NCORES)
    ]
    res = run_bass_kernel_spmd(nc, in_maps, core_ids=list(range(NCORES)), trace=trace)
    out = np.concatenate([res.results[i]["out"] for i in range(NCORES)], axis=0)
    if debug or trace:
        return out, res
    return out

